# revision 128
# baseline (speedup 1.0000x reference)
"""FourierLayer TRN2 kernel: folded DFT -> top-6 mask -> folded sparse inverse.

Contract: kernel(input_tensor=(8,2048,512) f32) -> (8,2048,512) f32.
Each of the 8 NeuronCores processes one batch element (data-parallel over
batch; no cross-core communication).

Cosine symmetry folding halves both DFT contractions:
  C[T-t,k] = C[t,k], S[T-t,k] = -S[t,k]  (C=cos, S=-sin of 2pi t k/T)
  u[t] = x[t]+x[T-t], v[t] = x[t]-x[T-t]   (host-side, free)
  Re[k] = sum_{t<=1024} Chalf[t,k] u[t]    (Chalf row 1024 = (-1)^k)
  Im[k] = sum_{t<1024}  Shalf[t,k] v[t]
  A[t]  = sum_k Ci[t,k] R2m[k]  (t<=1024),  B[t] = sum_k Si[t,k] I2m[k]
  out[t] = A+B, out[T-t] = A-B  (reflected half stored ascending; host
  flips out[1025:] at the end).

Forward is kc-major so magnitudes / transposes / top-k trickle during the
matmul stream; per (kc, chunk) the hi/lo product uses 3 matmuls (hi*hi,
hi*lo, lo*hi - the lo*lo term is below the top-6 selection noise floor).
Inverse matrices are single bf16 (only output amplitude, not selection).

Raw bass with manual semaphores. DMA semaphores are per-stream and
per-ring-slot-parity so every cumulative wait targets the LAST transfer
enqueued on that semaphore at wait time. (A shared counter is unsound:
each transfer increments once per SDMA engine in per-engine FIFO order,
but engines drift, so increments from a later enqueued transfer can
satisfy a wait while an earlier transfer is still in flight on a lagging
engine. This was observed as run-to-run top-k selection corruption.)
"""

from contextlib import ExitStack

import numpy as np
import ml_dtypes

import concourse.bass as bass
import concourse.mybir as mybir

BF16 = mybir.dt.bfloat16
F32 = mybir.dt.float32
AF = mybir.ActivationFunctionType
ALU = mybir.AluOpType

T = 2048
D = 512
KF = 1024
TH = 1024          # half length
NKC = KF // 128    # 8 freq chunks
NDC = D // 128     # 4 channel chunks
NCA = 9            # Re t-chunks (rows 0..1151, 1025+ zero)
NCB = 8            # Im t-chunks
TOPK = 6
WRE = NCA * 256    # Re stripe cols (9 a-tiles x [hi|lo])
NCF = 2 * NKC      # 16 forward stripes, order Re-k0, Im-k0, Re-k1, ...
NIV = 8            # inverse t-chunks (t=0..1023; row 1024 done on host)

# ---- semaphore schedule ----
# Semaphore values are cumulative in ENGINE EXECUTION ORDER.
# s_pe (tensor order: Re-k0, Im-k0, Re-k1, Im-k1, T0, Re-k2, Im-k2, T1,
#       ..., Re-k7, Im-k7, T6, T7, bcast, inv tc0..tc7):
#   Re-kc -> _RE(kc), Im-kc -> _IM(kc), T(kc) -> _TP(kc), bcast -> 25,
#   inv tc -> 26+tc (26..33)
# s_act (scalar order): r2-evict-kc -> 4kc+1, i2-evict-kc -> 4kc+2,
#   r2h-cast-kc -> 4kc+3, i2h-cast-kc -> 4kc+4 (1..32); thb -> 33;
#   A-evict tc -> 34+tc (34..41)
# s_dve (vector order: mag-k0, mag-k1, max8-k0, mag-k2, max8-k1, ...,
#       mag-k7, max8-k6, max8-k7, finalmax, mask, combines):
#   mag-kc -> _MG(kc); max8-kc -> _MX(kc); finalmax -> 17;
#   mask-kc -> 18+kc (18..25); combine lo-tc0 -> 26, hi-tc0 -> 27,
#   pmcopy -> 28; lo/hi-tcj (j>=1) -> 27+2j, 28+2j (.. 41, 42)
# s_pe inverse: tc0..3 -> 26..29, pmrow -> 30, tc4..7 -> 31..34
# s_pool: ones 1; ident 2
# DMA: s_ldu/s_ldu2/s_c0a (split startup loads), s_ldv (vh,vl),
#      s_cf[j%2] (16 stripes), s_iv (all 8 iv chunks, resident),
#      s_trow (4), s_out[tc%4] (2 per tc), s_ox (pm)


def _RE(kc):
    return 1 if kc == 0 else 3 * kc


def _IM(kc):
    return 2 if kc == 0 else 3 * kc + 1


def _TP(kc):
    return 24 if kc == 7 else 3 * kc + 5


def _MG(kc):
    return 1 if kc == 0 else 2 * kc


def _MX(kc):
    return 2 * kc + 3


def build_kernel(nc: bass.Bass):
    # u/v uploads pre-arranged host-side to [128, chunks*D] (contiguous
    # per-partition DMA lines instead of a 1KB-row gather)
    uh = nc.dram_tensor("uh", (128, NCA * D), BF16, kind="ExternalInput")
    ul = nc.dram_tensor("ul", (128, NCA * D), BF16, kind="ExternalInput")
    vh = nc.dram_tensor("vh", (128, NCB * D), BF16, kind="ExternalInput")
    vl = nc.dram_tensor("vl", (128, NCB * D), BF16, kind="ExternalInput")
    # forward stripes: [j, p, cols]; j=2kc -> Re stripe kc (9 a-tiles of
    # [hi 128 | lo 128]); j=2kc+1 -> Im stripe kc (8 a-tiles, padded)
    cf = nc.dram_tensor("cf", (NCF, 128, WRE), BF16, kind="ExternalInput")
    # inverse blocks per t-chunk: [tc, p, 2*KF] = [CiT | SiT], kc-major
    iv = nc.dram_tensor("iv", (NIV, 128, 2 * KF), BF16, kind="ExternalInput")
    # (-1)^(p+1) column for the out[1024] row reduction
    pm = nc.dram_tensor("pm", (128, 1), BF16, kind="ExternalInput")
    # bf16 output (host upcasts); halves store traffic
    out = nc.dram_tensor("out", (T, D), BF16, kind="ExternalOutput")

    with ExitStack() as ctx:
        def sb(name, shape, dtype):
            return ctx.enter_context(nc.sbuf_tensor(name, shape, dtype))

        uh_sb = sb("uh_sb", [128, NCA * D], BF16)
        ul_sb = sb("ul_sb", [128, NCA * D], BF16)
        vh_sb = sb("vh_sb", [128, NCB * D], BF16)
        vl_sb = sb("vl_sb", [128, NCB * D], BF16)
        cf_sb = sb("cf_sb", [128, 2 * WRE], BF16)
        iv_sb = sb("iv_sb", [128, NIV * 2 * KF], BF16)  # all chunks resident
        r2 = sb("r2", [128, NKC * D], F32)
        i2 = sb("i2", [128, NKC * D], F32)
        r2h = sb("r2h", [128, NKC * D], BF16)
        i2h = sb("i2h", [128, NKC * D], BF16)
        mag = sb("mag", [128, NKC * D], F32)
        m8i = sb("m8i", [128, NDC * 64], F32)   # per-kc top8 candidates
        m8f = sb("m8f", [128, NDC * 8], F32)    # final top8 per dc
        trows = sb("trows", [1, D], F32)
        thb = sb("thb", [128, D], F32)
        ones = sb("ones", [1, 128], F32)
        ident = sb("ident", [128, 128], F32)
        msk = sb("msk", [128, D], BF16)
        sqt = sb("sqt", [128, D], F32)
        ot_lo = sb("ot_lo", [128, 4 * D], BF16)
        ot_hi = sb("ot_hi", [128, 4 * D], BF16)
        ab_sb = sb("ab_sb", [128, 4 * D], F32)   # A evictions (4-slot ring)
        pm_sb = sb("pm_sb", [128, 1], BF16)
        banks = [ctx.enter_context(nc.psum_tensor(f"pb{i}", [128, D], F32))
                 for i in range(8)]
        s_ldu = ctx.enter_context(nc.semaphore())
        s_ldu2 = ctx.enter_context(nc.semaphore())
        s_c0a = ctx.enter_context(nc.semaphore())
        s_ldv = ctx.enter_context(nc.semaphore())
        s_cf = [ctx.enter_context(nc.semaphore(name=f"s_cf{i}"))
                for i in range(2)]
        s_iv = ctx.enter_context(nc.semaphore())
        s_trow = ctx.enter_context(nc.semaphore())
        s_out = [ctx.enter_context(nc.semaphore(name=f"s_out{i}"))
                 for i in range(4)]
        s_ox = ctx.enter_context(nc.semaphore())
        s_pe = ctx.enter_context(nc.semaphore())
        s_act = ctx.enter_context(nc.semaphore())
        s_dve = ctx.enter_context(nc.semaphore())
        s_pool = ctx.enter_context(nc.semaphore())
        block = ctx.enter_context(nc.Block())

        @block.gpsimd
        def _(gpsimd):
            # startup-critical loads first, split so the first matmul trios
            # start on partial data; later loads are deferred so they don't
            # steal DMA bandwidth from the critical path
            SP = 3 * D
            gpsimd.dma_start(uh_sb[:, 0:SP], uh[:, 0:SP]).then_inc(s_ldu, 16)
            gpsimd.dma_start(ul_sb[:, 0:SP], ul[:, 0:SP]).then_inc(s_ldu, 16)
            gpsimd.dma_start(cf_sb[:, 0:768], cf[0, :, 0:768]).then_inc(s_c0a, 16)
            gpsimd.dma_start(uh_sb[:, SP:], uh[:, SP:]).then_inc(s_ldu2, 16)
            gpsimd.dma_start(ul_sb[:, SP:], ul[:, SP:]).then_inc(s_ldu2, 16)
            gpsimd.dma_start(cf_sb[:, 768:WRE],
                             cf[0, :, 768:WRE]).then_inc(s_cf[0], 16)
            # constants
            gpsimd.memset(ones[:], 1.0).then_inc(s_pool, 1)
            gpsimd.memset(ident[:], 0.0)
            gpsimd.drain()
            nc.gpsimd.affine_select(
                out=ident[:], in_=ident[:],
                compare_op=ALU.not_equal, fill=1.0, base=0,
                pattern=[[-1, 128]], channel_multiplier=1,
            ).then_inc(s_pool, 1)
            gpsimd.dma_start(cf_sb[:, WRE:2 * WRE],
                             cf[1, :, :]).then_inc(s_cf[1], 16)
            gpsimd.dma_start(vh_sb[:, :], vh[:, :]).then_inc(s_ldv, 16)
            gpsimd.dma_start(vl_sb[:, :], vl[:, :]).then_inc(s_ldv, 16)
            gpsimd.dma_start(pm_sb[:, :], pm[:, :]).then_inc(s_ox, 16)
            # remaining forward stripes, ring slot j%2, gated 2 behind;
            # iv prefetches slipped in once the startup burst has drained
            for j in range(2, NCF):
                kcp, php = divmod(j - 2, 2)
                gpsimd.wait_ge(s_pe, _IM(kcp) if php else _RE(kcp))
                gpsimd.dma_start(
                    cf_sb[:, (j % 2) * WRE:(j % 2 + 1) * WRE],
                    cf[j, :, :]).then_inc(s_cf[j % 2], 16)
                if 4 <= j <= 11:
                    jj = j - 4
                    gpsimd.dma_start(
                        iv_sb[:, jj * 2 * KF:(jj + 1) * 2 * KF],
                        iv[jj, :, :]).then_inc(s_iv, 16)
            # theta rows: m8f col (dc*8+5) [128,1] -> trows [1,128] segment
            # (partition->free move; DMA matches flat iteration order)
            gpsimd.wait_ge(s_dve, 17)
            for dc in range(NDC):
                gpsimd.dma_start(
                    trows[0:1, dc * 128:(dc + 1) * 128],
                    m8f[:, dc * 8 + TOPK - 1:dc * 8 + TOPK],
                ).then_inc(s_trow, 16)
            # output stores
            def hi_inc(tc):
                return 28 if tc == 0 else 28 + 2 * tc

            for tcb in range(NIV):
                gpsimd.wait_ge(s_dve, 26 if tcb == 0 else hi_inc(tcb) - 1)
                gpsimd.dma_start(
                    out[tcb * 128:(tcb + 1) * 128, :],
                    ot_lo[:, (tcb % 4) * D:(tcb % 4 + 1) * D],
                ).then_inc(s_out[tcb % 4], 16)
                gpsimd.wait_ge(s_dve, hi_inc(tcb))
                # hi chunk tc0 row 0 carries out[1024] (pmcopy)
                gpsimd.dma_start(
                    out[TH + tcb * 128:TH + (tcb + 1) * 128, :],
                    ot_hi[:, (tcb % 4) * D:(tcb % 4 + 1) * D],
                ).then_inc(s_out[tcb % 4], 16)
            gpsimd.wait_ge(s_ldu, 32)
            gpsimd.wait_ge(s_ldu2, 32)
            gpsimd.wait_ge(s_c0a, 16)
            gpsimd.wait_ge(s_ldv, 32)
            gpsimd.wait_ge(s_cf[0], 128)
            gpsimd.wait_ge(s_cf[1], 128)
            gpsimd.wait_ge(s_iv, 128)
            for q in range(4):
                gpsimd.wait_ge(s_out[q], 64)
            gpsimd.wait_ge(s_trow, 64)
            gpsimd.wait_ge(s_ox, 16)

        @block.tensor
        def _(tensor):
            def fwd_group(ph, kc, mh_sb, ml_sb, ncc):
                j = 2 * kc + ph
                bank = banks[(kc % 4) * 2 + ph]
                if j == 0:
                    tensor.wait_ge(s_c0a, 16)   # first 3 a-tiles of stripe 0
                else:
                    tensor.wait_ge(s_cf[ph], 16 * (kc + 1))
                base = (j % 2) * WRE
                for a in range(ncc):
                    if j == 0 and a == 3:
                        tensor.wait_ge(s_ldu2, 32)
                        tensor.wait_ge(s_cf[0], 16)
                    hi = cf_sb[:, base + a * 256:base + a * 256 + 128]
                    lo = cf_sb[:, base + a * 256 + 128:base + a * 256 + 256]
                    xh_c = mh_sb[:, a * D:(a + 1) * D]
                    xl_c = ml_sb[:, a * D:(a + 1) * D]
                    last = (a == ncc - 1)
                    nc.tensor.matmul(bank[:], hi, xh_c,
                                     start=(a == 0), stop=False)
                    nc.tensor.matmul(bank[:], hi, xl_c,
                                     start=False, stop=False)
                    mm = nc.tensor.matmul(bank[:], lo, xh_c,
                                          start=False, stop=last)
                    if last:
                        mm.then_inc(s_pe, 1)

            def transposes(kc):
                # 4 transposes of mag chunk kc into bank (kc%4)*2
                tensor.wait_ge(s_dve, _MG(kc))
                tensor.wait_ge(s_act, 4 * kc + 1)
                b = banks[(kc % 4) * 2]
                for dc in range(NDC):
                    mm = nc.tensor.transpose(
                        b[:, dc * 128:(dc + 1) * 128],
                        mag[:, kc * D + dc * 128:kc * D + (dc + 1) * 128],
                        ident[:])
                    if dc == NDC - 1:
                        mm.then_inc(s_pe, 1)

            tensor.wait_ge(s_ldu, 32)
            tensor.wait_ge(s_pool, 2)
            for kc in range(NKC):
                if kc >= 4:
                    tensor.wait_ge(s_dve, _MX(kc - 4))  # max8-(kc-4): bank
                fwd_group(0, kc, uh_sb, ul_sb, NCA)
                if kc == 0:
                    tensor.wait_ge(s_ldv, 32)
                if kc >= 4:
                    tensor.wait_ge(s_act, 4 * (kc - 4) + 2)  # i2-evict(kc-4)
                fwd_group(1, kc, vh_sb, vl_sb, NCB)
                if kc >= 1:
                    transposes(kc - 1)
            transposes(NKC - 1)
            # theta broadcast: ones^T (1,128) x trows (1,512) -> thb psum
            tensor.wait_ge(s_trow, 64)
            nc.tensor.matmul(banks[7][:], ones[:], trows[:],
                             start=True, stop=True).then_inc(s_pe, 1)
            # inverse: per tc, A into banks[(tc%2)*2] from r2h,
            #          B into banks[(tc%2)*2+1] from i2h
            # tc0..tc3 interleaved per kc (all 8 banks), paced by the masks
            tensor.wait_ge(s_iv, 16 * NIV)   # all iv chunks resident
            for kc in range(NKC):
                tensor.wait_ge(s_dve, 18 + kc)  # mask-kc (masked r2h/i2h)
                dsl = slice(kc * D, (kc + 1) * D)
                for tcb in range(4):
                    sl0 = tcb * 2 * KF
                    csl = slice(sl0 + kc * 128, sl0 + (kc + 1) * 128)
                    ssl = slice(sl0 + KF + kc * 128, sl0 + KF + (kc + 1) * 128)
                    nc.tensor.matmul(banks[tcb * 2][:], iv_sb[:, csl],
                                     r2h[:, dsl],
                                     start=(kc == 0), stop=(kc == NKC - 1))
                    mm = nc.tensor.matmul(
                        banks[tcb * 2 + 1][:], iv_sb[:, ssl], i2h[:, dsl],
                        start=(kc == 0), stop=(kc == NKC - 1))
                    if kc == NKC - 1:
                        mm.then_inc(s_pe, 1)  # tc0..3 -> 26..29
            # out[1024] row: sum_k (-1)^k R2m[k] into banks[0] row 0
            # (banks[0] freed by A-evict tc0; read by pmcopy before tc4)
            tensor.wait_ge(s_ox, 16)
            tensor.wait_ge(s_act, 34)   # A-evict tc0
            for kc in range(NKC):
                mm = nc.tensor.matmul(
                    banks[0][0:1, :], pm_sb[:, :],
                    r2h[:, kc * D:(kc + 1) * D],
                    start=(kc == 0), stop=(kc == NKC - 1))
            mm.then_inc(s_pe, 1)  # pmrow -> 30
            # remaining inverse chunks
            for tcb in range(4, NIV):
                tensor.wait_ge(
                    s_dve, 28 if tcb == 4 else 28 + 2 * (tcb - 4))
                bA = banks[(tcb % 4) * 2]
                bB = banks[(tcb % 4) * 2 + 1]
                sl0 = tcb * 2 * KF
                for kc in range(NKC):
                    dsl = slice(kc * D, (kc + 1) * D)
                    csl = slice(sl0 + kc * 128, sl0 + (kc + 1) * 128)
                    ssl = slice(sl0 + KF + kc * 128, sl0 + KF + (kc + 1) * 128)
                    nc.tensor.matmul(bA[:], iv_sb[:, csl], r2h[:, dsl],
                                     start=(kc == 0), stop=(kc == NKC - 1))
                    mm = nc.tensor.matmul(
                        bB[:], iv_sb[:, ssl], i2h[:, dsl],
                        start=(kc == 0), stop=(kc == NKC - 1))
                    if kc == NKC - 1:
                        mm.then_inc(s_pe, 1)  # tc4..7 -> 31..34

        @block.scalar
        def _(scalar):
            # forward evictions; x2 scale folds the conjugate doubling
            for kc in range(NKC):
                dsl = slice(kc * D, (kc + 1) * D)
                scalar.wait_ge(s_pe, _RE(kc))
                nc.scalar.activation(
                    r2[:, dsl], banks[(kc % 4) * 2][:],
                    AF.Copy, scale=2.0).then_inc(s_act, 1)
                scalar.wait_ge(s_pe, _IM(kc))
                nc.scalar.activation(
                    i2[:, dsl], banks[(kc % 4) * 2 + 1][:],
                    AF.Copy, scale=2.0).then_inc(s_act, 1)
                if kc == NKC - 1:
                    # thb ahead of the k7 casts: it gates the whole mask
                    # pipeline, the casts only gate mask round k7
                    scalar.wait_ge(s_pe, 25)
                    nc.scalar.activation(thb[:], banks[7][:],
                                         AF.Copy).then_inc(s_act, 1)
                # pre-cast (unmasked); the mask is applied in bf16 on vector
                nc.scalar.activation(r2h[:, dsl], r2[:, dsl],
                                     AF.Copy).then_inc(s_act, 1)
                nc.scalar.activation(i2h[:, dsl], i2[:, dsl],
                                     AF.Copy).then_inc(s_act, 1)
            # inverse A evictions (psum -> sbuf ring; frees the 2-psum-input
            # restriction for the vector A+B / A-B combines)
            for tcb in range(8):
                scalar.wait_ge(s_pe, 26 + tcb if tcb <= 3 else 27 + tcb)
                if tcb >= 4:   # ab slot (4-ring) read by combines of tcb-4
                    scalar.wait_ge(
                        s_dve, 27 if tcb == 4 else 28 + 2 * (tcb - 4))
                nc.scalar.activation(
                    ab_sb[:, (tcb % 4) * D:(tcb % 4 + 1) * D],
                    banks[(tcb % 4) * 2][:], AF.Copy).then_inc(s_act, 1)


        @block.vector
        def _(vector):
            # magnitudes + incremental top-8 (interleaved, max8 lags 1 kc)
            def mag_kc(kc):
                vector.wait_ge(s_act, 4 * kc + 2)
                dsl = slice(kc * D, (kc + 1) * D)
                nc.vector.tensor_tensor(mag[:, dsl], r2[:, dsl], r2[:, dsl],
                                        ALU.mult)
                nc.vector.tensor_tensor(sqt[:], i2[:, dsl], i2[:, dsl],
                                        ALU.mult)
                nc.vector.tensor_tensor(mag[:, dsl], mag[:, dsl], sqt[:],
                                        ALU.add).then_inc(s_dve, 1)

            def max8_kc(kc):
                vector.wait_ge(s_pe, _TP(kc))
                b = banks[(kc % 4) * 2]
                for dc in range(NDC):
                    mx = nc.vector.max(
                        out=m8i[:, dc * 64 + kc * 8:dc * 64 + (kc + 1) * 8],
                        in_=b[:, dc * 128:(dc + 1) * 128])
                    if dc == NDC - 1:
                        mx.then_inc(s_dve, 1)

            mag_kc(0)
            for kc in range(1, NKC):
                mag_kc(kc)
                max8_kc(kc - 1)
            max8_kc(NKC - 1)
            for dc in range(NDC):
                mx = nc.vector.max(out=m8f[:, dc * 8:(dc + 1) * 8],
                                   in_=m8i[:, dc * 64:(dc + 1) * 64])
                if dc == NDC - 1:
                    mx.then_inc(s_dve, 1)
            # mask + apply to the pre-cast bf16 coefficients (in place)
            vector.wait_ge(s_act, 31)   # thb
            for kc in range(NKC):
                if kc == NKC - 1:
                    vector.wait_ge(s_act, 33)   # k7 casts
                dsl = slice(kc * D, (kc + 1) * D)
                nc.vector.tensor_tensor(msk[:], mag[:, dsl], thb[:], ALU.is_ge)
                nc.vector.tensor_tensor(r2h[:, dsl], r2h[:, dsl], msk[:],
                                        ALU.mult)
                nc.vector.tensor_tensor(i2h[:, dsl], i2h[:, dsl], msk[:],
                                        ALU.mult).then_inc(s_dve, 1)
            # inverse combines: lo = A+B, hi = A-B (A from sbuf, B from psum)
            for tcb in range(8):
                vector.wait_ge(s_act, 34 + tcb)   # A evicted (implies B done)
                if tcb >= 4:
                    vector.wait_ge(s_out[tcb % 4], 32 * (tcb // 4))
                bB = banks[(tcb % 4) * 2 + 1]
                asl = ab_sb[:, (tcb % 4) * D:(tcb % 4 + 1) * D]
                osl = slice((tcb % 4) * D, (tcb % 4 + 1) * D)
                nc.vector.tensor_tensor(ot_lo[:, osl], asl, bB[:],
                                        ALU.add).then_inc(s_dve, 1)
                nc.vector.tensor_tensor(ot_hi[:, osl], asl, bB[:],
                                        ALU.subtract).then_inc(s_dve, 1)
                if tcb == 0:
                    # out[1024] into ot_hi slot-0 row 0 (from pmrow psum)
                    vector.wait_ge(s_pe, 30)
                    nc.vector.tensor_copy(
                        ot_hi[0:1, 0:D], banks[0][0:1, :]).then_inc(s_dve, 1)


# ---------------- host side ----------------

_BF = ml_dtypes.bfloat16


def _split_hilo(a32):
    hi = a32.astype(_BF)
    lo = (a32 - hi.astype(np.float32)).astype(_BF)
    return hi, lo


def _make_constants():
    t = np.arange(T, dtype=np.float64)[:, None]
    k = np.arange(1, KF + 1, dtype=np.float64)[None, :]
    ang = 2.0 * np.pi * t * k / T
    C = np.cos(ang)
    S = -np.sin(ang)
    C[:, KF - 1] = 0.0
    S[:, KF - 1] = 0.0

    # folded forward halves
    Chalf = np.zeros((NCA * 128, KF))
    Chalf[:TH] = C[:TH]
    Chalf[TH] = np.cos(np.pi * k[0])
    Chalf[TH, KF - 1] = 0.0
    Shalf = np.zeros((NCB * 128, KF))
    Shalf[:] = S[:TH]

    def stripes(m64, ncc):
        hi, lo = _split_hilo(m64.astype(np.float32))
        # [a*128+p, kc*128+u] -> [kc, p, a, {hi|lo}, u]
        def tile(m):
            b = np.asarray(m, dtype=np.float32).reshape(ncc, 128, NKC, 128)
            return b.transpose(2, 1, 0, 3)             # (kc, p, a, u)
        st = np.stack([tile(hi), tile(lo)], axis=3)    # (kc, p, a, 2, u)
        st = st.reshape(NKC, 128, ncc * 256)
        if ncc < NCA:
            pad = np.zeros((NKC, 128, (NCA - ncc) * 256), np.float32)
            st = np.concatenate([st, pad], axis=2)
        return st

    cre = stripes(Chalf, NCA)
    cim = stripes(Shalf, NCB)
    cfc = np.empty((NCF, 128, WRE), np.float32)
    cfc[0::2] = cre
    cfc[1::2] = cim
    cfc = cfc.astype(_BF)

    # inverse blocks (single bf16), t = 0..1023 (row 1024 done on host)
    Ci = C[:TH].astype(np.float32)
    Si = S[:TH].astype(np.float32)

    def blocks(m32):
        M = np.ascontiguousarray(m32.T)                  # (KF, 1024)
        blk = M.reshape(NKC, 128, NIV, 128)              # (kc, p, tc, u)
        blk = np.ascontiguousarray(blk.transpose(2, 1, 0, 3))
        return blk.reshape(NIV, 128, KF)

    ivc = np.ascontiguousarray(
        np.concatenate([blocks(Ci), blocks(Si)], axis=2)).astype(_BF)
    pmc = ((-1.0) ** (np.arange(128) + 1)).astype(_BF)[:, None]
    return dict(cf=np.ascontiguousarray(cfc), iv=ivc, pm=pmc)


_CONSTS = None
LAST_EXEC_NS = None
LAST_RES = None
TRACE = False


def kernel(input_tensor: np.ndarray) -> np.ndarray:
    from concourse.bass_utils import run_bass_kernel_spmd

    global _CONSTS
    if _CONSTS is None:
        _CONSTS = _make_constants()

    x = np.asarray(input_tensor, dtype=np.float32)
    B = x.shape[0]
    assert x.shape == (B, T, D)

    nc = bass.Bass("TRN2", target_bir_lowering=False)
    build_kernel(nc)

    in_maps = []
    for b in range(B):
        xb = x[b].astype(np.float64)
        u = np.zeros((NCA * 128, D))
        v = np.zeros((NCB * 128, D))
        u[0] = xb[0]
        u[1:TH] = xb[1:TH] + xb[T - 1:TH:-1]
        u[TH] = xb[TH]
        v[1:TH] = xb[1:TH] - xb[T - 1:TH:-1]

        def pre(m, ncc):   # [a*128+p, d] -> [p, a*D+d] (contiguous DMA)
            return np.ascontiguousarray(
                m.reshape(ncc, 128, D).transpose(1, 0, 2).reshape(128, ncc * D))

        uh_np, ul_np = _split_hilo(u.astype(np.float32))
        vh_np, vl_np = _split_hilo(v.astype(np.float32))
        in_maps.append({"uh": pre(uh_np, NCA), "ul": pre(ul_np, NCA),
                        "vh": pre(vh_np, NCB), "vl": pre(vl_np, NCB),
                        **_CONSTS})

    global LAST_EXEC_NS, LAST_RES
    res = run_bass_kernel_spmd(nc, in_maps, core_ids=list(range(B)), trace=TRACE)
    LAST_EXEC_NS = res.exec_time_ns
    LAST_RES = res
    outs = []
    for b in range(B):
        y = res.results[b]["out"].astype(np.float32)
        y[TH + 1:] = y[TH + 1:][::-1]   # unreverse the reflected half
        outs.append(y)
    return np.stack(outs, axis=0)


if __name__ == "__main__":
    rng = np.random.default_rng(0)
    x = rng.standard_normal((8, T, D), dtype=np.float32)
    y = kernel(input_tensor=x)
    print("out", y.shape, y.dtype)


# revision 129
# speedup vs baseline: 1.0198x; 1.0198x over previous
"""FourierLayer TRN2 kernel: folded DFT -> top-6 mask -> folded sparse inverse.

Contract: kernel(input_tensor=(8,2048,512) f32) -> (8,2048,512) f32.
Each of the 8 NeuronCores processes one batch element (data-parallel over
batch; no cross-core communication).

Cosine symmetry folding halves both DFT contractions:
  C[T-t,k] = C[t,k], S[T-t,k] = -S[t,k]  (C=cos, S=-sin of 2pi t k/T)
  u[t] = x[t]+x[T-t], v[t] = x[t]-x[T-t]   (host-side, free)
  Re[k] = sum_{t<=1024} Chalf[t,k] u[t]    (Chalf row 1024 = (-1)^k)
  Im[k] = sum_{t<1024}  Shalf[t,k] v[t]
  A[t]  = sum_k Ci[t,k] R2m[k]  (t<=1024),  B[t] = sum_k Si[t,k] I2m[k]
  out[t] = A+B, out[T-t] = A-B  (reflected half stored ascending; host
  flips out[1025:] at the end).

Forward is kc-major so magnitudes / transposes / top-k trickle during the
matmul stream; per (kc, chunk) the hi/lo product uses 3 matmuls (hi*hi,
hi*lo, lo*hi - the lo*lo term is below the top-6 selection noise floor).
Inverse matrices are single bf16 (only output amplitude, not selection).

Raw bass with manual semaphores. DMA semaphores are per-stream and
per-ring-slot-parity so every cumulative wait targets the LAST transfer
enqueued on that semaphore at wait time. (A shared counter is unsound:
each transfer increments once per SDMA engine in per-engine FIFO order,
but engines drift, so increments from a later enqueued transfer can
satisfy a wait while an earlier transfer is still in flight on a lagging
engine. This was observed as run-to-run top-k selection corruption.)
"""

from contextlib import ExitStack

import numpy as np
import ml_dtypes

import concourse.bass as bass
import concourse.mybir as mybir

BF16 = mybir.dt.bfloat16
F32 = mybir.dt.float32
AF = mybir.ActivationFunctionType
ALU = mybir.AluOpType

T = 2048
D = 512
KF = 1024
TH = 1024          # half length
NKC = KF // 128    # 8 freq chunks
NDC = D // 128     # 4 channel chunks
NCA = 9            # Re t-chunks (rows 0..1151, 1025+ zero)
NCB = 8            # Im t-chunks
TOPK = 6
WRE = NCA * 256    # Re stripe cols (9 a-tiles x [hi|lo])
NCF = 2 * NKC      # 16 forward stripes, order Re-k0, Im-k0, Re-k1, ...
NIV = 8            # inverse t-chunks (t=0..1023; row 1024 done on host)

# ---- semaphore schedule ----
# Semaphore values are cumulative in ENGINE EXECUTION ORDER.
# s_pe (tensor order: Re-k0, Im-k0, Re-k1, Im-k1, T0, Re-k2, Im-k2, T1,
#       ..., Re-k7, Im-k7, T6, T7, bcast, inv tc0..tc7):
#   Re-kc -> _RE(kc), Im-kc -> _IM(kc), T(kc) -> _TP(kc), bcast -> 25,
#   inv tc -> 26+tc (26..33)
# s_act (scalar order): r2-evict-kc -> 4kc+1, i2-evict-kc -> 4kc+2,
#   r2h-cast-kc -> 4kc+3, i2h-cast-kc -> 4kc+4 (1..32); thb -> 33;
#   A-evict tc -> 34+tc (34..41)
# s_dve (vector order: mag-k0, mag-k1, max8-k0, mag-k2, max8-k1, ...,
#       mag-k7, max8-k6, max8-k7, finalmax, mask, combines):
#   mag-kc -> _MG(kc); max8-kc -> _MX(kc); finalmax -> 17;
#   mask-kc -> 18+kc (18..25); combine lo-tc0 -> 26, hi-tc0 -> 27,
#   pmcopy -> 28; lo/hi-tcj (j>=1) -> 27+2j, 28+2j (.. 41, 42)
# s_pe inverse: tc0..3 -> 26..29, pmrow -> 30, tc4..7 -> 31..34
# s_pool: ones 1; ident 2
# DMA: s_ldu/s_ldu2/s_c0a (split startup loads), s_ldv (vh,vl),
#      s_cf[j%2] (16 stripes), s_iv (all 8 iv chunks, resident),
#      s_trow (4), s_out[tc%4] (2 per tc), s_ox (pm)


def _RE(kc):
    return 1 if kc == 0 else 3 * kc


def _IM(kc):
    return 2 if kc == 0 else 3 * kc + 1


def _TP(kc):
    return 24 if kc == 7 else 3 * kc + 5


def _MG(kc):
    return 1 if kc == 0 else 2 * kc


def _MX(kc):
    return 2 * kc + 3


def build_kernel(nc: bass.Bass):
    # u/v uploads pre-arranged host-side to [128, chunks*D] (contiguous
    # per-partition DMA lines instead of a 1KB-row gather)
    uh = nc.dram_tensor("uh", (128, NCA * D), BF16, kind="ExternalInput")
    ul = nc.dram_tensor("ul", (128, NCA * D), BF16, kind="ExternalInput")
    vh = nc.dram_tensor("vh", (128, NCB * D), BF16, kind="ExternalInput")
    vl = nc.dram_tensor("vl", (128, NCB * D), BF16, kind="ExternalInput")
    # forward stripes: [j, p, cols]; j=2kc -> Re stripe kc (9 a-tiles of
    # [hi 128 | lo 128]); j=2kc+1 -> Im stripe kc (8 a-tiles, padded)
    cf = nc.dram_tensor("cf", (NCF, 128, WRE), BF16, kind="ExternalInput")
    # inverse blocks per t-chunk: [tc, p, 2*KF] = [CiT | SiT], kc-major
    iv = nc.dram_tensor("iv", (NIV, 128, 2 * KF), BF16, kind="ExternalInput")
    # (-1)^(p+1) column for the out[1024] row reduction
    pm = nc.dram_tensor("pm", (128, 1), BF16, kind="ExternalInput")
    # bf16 output (host upcasts); halves store traffic
    out = nc.dram_tensor("out", (T, D), BF16, kind="ExternalOutput")

    with ExitStack() as ctx:
        def sb(name, shape, dtype):
            return ctx.enter_context(nc.sbuf_tensor(name, shape, dtype))

        uh_sb = sb("uh_sb", [128, NCA * D], BF16)
        ul_sb = sb("ul_sb", [128, NCA * D], BF16)
        vh_sb = sb("vh_sb", [128, NCB * D], BF16)
        vl_sb = sb("vl_sb", [128, NCB * D], BF16)
        cf_sb = sb("cf_sb", [128, 2 * WRE], BF16)
        iv_sb = sb("iv_sb", [128, NIV * 2 * KF], BF16)  # all chunks resident
        r2 = sb("r2", [128, NKC * D], F32)
        i2 = sb("i2", [128, NKC * D], F32)
        r2h = sb("r2h", [128, NKC * D], BF16)
        i2h = sb("i2h", [128, NKC * D], BF16)
        mag = sb("mag", [128, NKC * D], F32)
        m8i = sb("m8i", [128, NDC * 64], F32)   # per-kc top8 candidates
        m8f = sb("m8f", [128, NDC * 8], F32)    # final top8 per dc
        trows = sb("trows", [1, D], F32)
        thb = sb("thb", [128, D], F32)
        ones = sb("ones", [1, 128], F32)
        ident = sb("ident", [128, 128], F32)
        msk = sb("msk", [128, D], BF16)
        sqt = sb("sqt", [128, D], F32)
        ot_lo = sb("ot_lo", [128, 4 * D], BF16)
        ot_hi = sb("ot_hi", [128, 4 * D], BF16)
        ab_sb = sb("ab_sb", [128, 4 * D], F32)   # A evictions (4-slot ring)
        pm_sb = sb("pm_sb", [128, 1], BF16)
        banks = [ctx.enter_context(nc.psum_tensor(f"pb{i}", [128, D], F32))
                 for i in range(8)]
        s_ldu = ctx.enter_context(nc.semaphore())
        s_ldu2 = ctx.enter_context(nc.semaphore())
        s_c0a = ctx.enter_context(nc.semaphore())
        s_ldv = ctx.enter_context(nc.semaphore())
        s_cf = [ctx.enter_context(nc.semaphore(name=f"s_cf{i}"))
                for i in range(2)]
        s_iv = ctx.enter_context(nc.semaphore())
        s_trow = ctx.enter_context(nc.semaphore())
        s_out = [ctx.enter_context(nc.semaphore(name=f"s_out{i}"))
                 for i in range(4)]
        s_ox = ctx.enter_context(nc.semaphore())
        s_pe = ctx.enter_context(nc.semaphore())
        s_act = ctx.enter_context(nc.semaphore())
        s_dve = ctx.enter_context(nc.semaphore())
        s_pool = ctx.enter_context(nc.semaphore())
        block = ctx.enter_context(nc.Block())

        @block.gpsimd
        def _(gpsimd):
            # startup-critical loads first, split so the first matmul trios
            # start on partial data; later loads are deferred so they don't
            # steal DMA bandwidth from the critical path
            SP = 3 * D
            gpsimd.dma_start(uh_sb[:, 0:SP], uh[:, 0:SP]).then_inc(s_ldu, 16)
            gpsimd.dma_start(ul_sb[:, 0:SP], ul[:, 0:SP]).then_inc(s_ldu, 16)
            gpsimd.dma_start(cf_sb[:, 0:768], cf[0, :, 0:768]).then_inc(s_c0a, 16)
            gpsimd.dma_start(uh_sb[:, SP:], uh[:, SP:]).then_inc(s_ldu2, 16)
            gpsimd.dma_start(ul_sb[:, SP:], ul[:, SP:]).then_inc(s_ldu2, 16)
            gpsimd.dma_start(cf_sb[:, 768:WRE],
                             cf[0, :, 768:WRE]).then_inc(s_cf[0], 16)
            # constants
            gpsimd.memset(ones[:], 1.0).then_inc(s_pool, 1)
            gpsimd.memset(ident[:], 0.0)
            gpsimd.drain()
            nc.gpsimd.affine_select(
                out=ident[:], in_=ident[:],
                compare_op=ALU.not_equal, fill=1.0, base=0,
                pattern=[[-1, 128]], channel_multiplier=1,
            ).then_inc(s_pool, 1)
            gpsimd.dma_start(cf_sb[:, WRE:2 * WRE],
                             cf[1, :, :]).then_inc(s_cf[1], 16)
            gpsimd.dma_start(vh_sb[:, :], vh[:, :]).then_inc(s_ldv, 16)
            gpsimd.dma_start(vl_sb[:, :], vl[:, :]).then_inc(s_ldv, 16)
            gpsimd.dma_start(pm_sb[:, :], pm[:, :]).then_inc(s_ox, 16)
            # remaining forward stripes, ring slot j%2, gated 2 behind;
            # iv prefetches slipped in once the startup burst has drained
            for j in range(2, NCF):
                kcp, php = divmod(j - 2, 2)
                gpsimd.wait_ge(s_pe, _IM(kcp) if php else _RE(kcp))
                gpsimd.dma_start(
                    cf_sb[:, (j % 2) * WRE:(j % 2 + 1) * WRE],
                    cf[j, :, :]).then_inc(s_cf[j % 2], 16)
                if 8 <= j <= 15:
                    jj = j - 8
                    gpsimd.dma_start(
                        iv_sb[:, jj * 2 * KF:(jj + 1) * 2 * KF],
                        iv[jj, :, :]).then_inc(s_iv, 16)
            # theta rows: m8f col (dc*8+5) [128,1] -> trows [1,128] segment
            # (partition->free move; DMA matches flat iteration order)
            gpsimd.wait_ge(s_dve, 17)
            for dc in range(NDC):
                gpsimd.dma_start(
                    trows[0:1, dc * 128:(dc + 1) * 128],
                    m8f[:, dc * 8 + TOPK - 1:dc * 8 + TOPK],
                ).then_inc(s_trow, 16)
            # output stores
            def hi_inc(tc):
                return 28 if tc == 0 else 28 + 2 * tc

            for tcb in range(NIV):
                gpsimd.wait_ge(s_dve, 26 if tcb == 0 else hi_inc(tcb) - 1)
                gpsimd.dma_start(
                    out[tcb * 128:(tcb + 1) * 128, :],
                    ot_lo[:, (tcb % 4) * D:(tcb % 4 + 1) * D],
                ).then_inc(s_out[tcb % 4], 16)
                gpsimd.wait_ge(s_dve, hi_inc(tcb))
                # hi chunk tc0 row 0 carries out[1024] (pmcopy)
                gpsimd.dma_start(
                    out[TH + tcb * 128:TH + (tcb + 1) * 128, :],
                    ot_hi[:, (tcb % 4) * D:(tcb % 4 + 1) * D],
                ).then_inc(s_out[tcb % 4], 16)
            gpsimd.wait_ge(s_ldu, 32)
            gpsimd.wait_ge(s_ldu2, 32)
            gpsimd.wait_ge(s_c0a, 16)
            gpsimd.wait_ge(s_ldv, 32)
            gpsimd.wait_ge(s_cf[0], 128)
            gpsimd.wait_ge(s_cf[1], 128)
            gpsimd.wait_ge(s_iv, 128)
            for q in range(4):
                gpsimd.wait_ge(s_out[q], 64)
            gpsimd.wait_ge(s_trow, 64)
            gpsimd.wait_ge(s_ox, 16)

        @block.tensor
        def _(tensor):
            def fwd_group(ph, kc, mh_sb, ml_sb, ncc):
                j = 2 * kc + ph
                bank = banks[(kc % 4) * 2 + ph]
                if j == 0:
                    tensor.wait_ge(s_c0a, 16)   # first 3 a-tiles of stripe 0
                else:
                    tensor.wait_ge(s_cf[ph], 16 * (kc + 1))
                base = (j % 2) * WRE
                for a in range(ncc):
                    if j == 0 and a == 3:
                        tensor.wait_ge(s_ldu2, 32)
                        tensor.wait_ge(s_cf[0], 16)
                    hi = cf_sb[:, base + a * 256:base + a * 256 + 128]
                    lo = cf_sb[:, base + a * 256 + 128:base + a * 256 + 256]
                    xh_c = mh_sb[:, a * D:(a + 1) * D]
                    xl_c = ml_sb[:, a * D:(a + 1) * D]
                    last = (a == ncc - 1)
                    nc.tensor.matmul(bank[:], hi, xh_c,
                                     start=(a == 0), stop=False)
                    nc.tensor.matmul(bank[:], hi, xl_c,
                                     start=False, stop=False)
                    mm = nc.tensor.matmul(bank[:], lo, xh_c,
                                          start=False, stop=last)
                    if last:
                        mm.then_inc(s_pe, 1)

            def transposes(kc):
                # 4 transposes of mag chunk kc into bank (kc%4)*2
                tensor.wait_ge(s_dve, _MG(kc))
                tensor.wait_ge(s_act, 4 * kc + 1)
                b = banks[(kc % 4) * 2]
                for dc in range(NDC):
                    mm = nc.tensor.transpose(
                        b[:, dc * 128:(dc + 1) * 128],
                        mag[:, kc * D + dc * 128:kc * D + (dc + 1) * 128],
                        ident[:])
                    if dc == NDC - 1:
                        mm.then_inc(s_pe, 1)

            tensor.wait_ge(s_ldu, 32)
            tensor.wait_ge(s_pool, 2)
            for kc in range(NKC):
                if kc >= 4:
                    tensor.wait_ge(s_dve, _MX(kc - 4))  # max8-(kc-4): bank
                fwd_group(0, kc, uh_sb, ul_sb, NCA)
                if kc == 0:
                    tensor.wait_ge(s_ldv, 32)
                if kc >= 4:
                    tensor.wait_ge(s_act, 4 * (kc - 4) + 2)  # i2-evict(kc-4)
                fwd_group(1, kc, vh_sb, vl_sb, NCB)
                if kc >= 1:
                    transposes(kc - 1)
            transposes(NKC - 1)
            # theta broadcast: ones^T (1,128) x trows (1,512) -> thb psum
            tensor.wait_ge(s_trow, 64)
            nc.tensor.matmul(banks[7][:], ones[:], trows[:],
                             start=True, stop=True).then_inc(s_pe, 1)
            # inverse: per tc, A into banks[(tc%2)*2] from r2h,
            #          B into banks[(tc%2)*2+1] from i2h
            # tc0..tc3 interleaved per kc (all 8 banks), paced by the masks
            tensor.wait_ge(s_iv, 16 * NIV)   # all iv chunks resident
            for kc in range(NKC):
                tensor.wait_ge(s_dve, 18 + kc)  # mask-kc (masked r2h/i2h)
                dsl = slice(kc * D, (kc + 1) * D)
                for tcb in range(4):
                    sl0 = tcb * 2 * KF
                    csl = slice(sl0 + kc * 128, sl0 + (kc + 1) * 128)
                    ssl = slice(sl0 + KF + kc * 128, sl0 + KF + (kc + 1) * 128)
                    nc.tensor.matmul(banks[tcb * 2][:], iv_sb[:, csl],
                                     r2h[:, dsl],
                                     start=(kc == 0), stop=(kc == NKC - 1))
                    mm = nc.tensor.matmul(
                        banks[tcb * 2 + 1][:], iv_sb[:, ssl], i2h[:, dsl],
                        start=(kc == 0), stop=(kc == NKC - 1))
                    if kc == NKC - 1:
                        mm.then_inc(s_pe, 1)  # tc0..3 -> 26..29
            # out[1024] row: sum_k (-1)^k R2m[k] into banks[0] row 0
            # (banks[0] freed by A-evict tc0; read by pmcopy before tc4)
            tensor.wait_ge(s_ox, 16)
            tensor.wait_ge(s_act, 34)   # A-evict tc0
            for kc in range(NKC):
                mm = nc.tensor.matmul(
                    banks[0][0:1, :], pm_sb[:, :],
                    r2h[:, kc * D:(kc + 1) * D],
                    start=(kc == 0), stop=(kc == NKC - 1))
            mm.then_inc(s_pe, 1)  # pmrow -> 30
            # remaining inverse chunks
            for tcb in range(4, NIV):
                tensor.wait_ge(
                    s_dve, 28 if tcb == 4 else 28 + 2 * (tcb - 4))
                bA = banks[(tcb % 4) * 2]
                bB = banks[(tcb % 4) * 2 + 1]
                sl0 = tcb * 2 * KF
                for kc in range(NKC):
                    dsl = slice(kc * D, (kc + 1) * D)
                    csl = slice(sl0 + kc * 128, sl0 + (kc + 1) * 128)
                    ssl = slice(sl0 + KF + kc * 128, sl0 + KF + (kc + 1) * 128)
                    nc.tensor.matmul(bA[:], iv_sb[:, csl], r2h[:, dsl],
                                     start=(kc == 0), stop=(kc == NKC - 1))
                    mm = nc.tensor.matmul(
                        bB[:], iv_sb[:, ssl], i2h[:, dsl],
                        start=(kc == 0), stop=(kc == NKC - 1))
                    if kc == NKC - 1:
                        mm.then_inc(s_pe, 1)  # tc4..7 -> 31..34

        @block.scalar
        def _(scalar):
            # forward evictions; x2 scale folds the conjugate doubling
            for kc in range(NKC):
                dsl = slice(kc * D, (kc + 1) * D)
                scalar.wait_ge(s_pe, _RE(kc))
                nc.scalar.activation(
                    r2[:, dsl], banks[(kc % 4) * 2][:],
                    AF.Copy, scale=2.0).then_inc(s_act, 1)
                scalar.wait_ge(s_pe, _IM(kc))
                nc.scalar.activation(
                    i2[:, dsl], banks[(kc % 4) * 2 + 1][:],
                    AF.Copy, scale=2.0).then_inc(s_act, 1)
                if kc == NKC - 1:
                    # thb ahead of the k7 casts: it gates the whole mask
                    # pipeline, the casts only gate mask round k7
                    scalar.wait_ge(s_pe, 25)
                    nc.scalar.activation(thb[:], banks[7][:],
                                         AF.Copy).then_inc(s_act, 1)
                # pre-cast (unmasked); the mask is applied in bf16 on vector
                nc.scalar.activation(r2h[:, dsl], r2[:, dsl],
                                     AF.Copy).then_inc(s_act, 1)
                nc.scalar.activation(i2h[:, dsl], i2[:, dsl],
                                     AF.Copy).then_inc(s_act, 1)
            # inverse A evictions (psum -> sbuf ring; frees the 2-psum-input
            # restriction for the vector A+B / A-B combines)
            for tcb in range(8):
                scalar.wait_ge(s_pe, 26 + tcb if tcb <= 3 else 27 + tcb)
                if tcb >= 4:   # ab slot (4-ring) read by combines of tcb-4
                    scalar.wait_ge(
                        s_dve, 27 if tcb == 4 else 28 + 2 * (tcb - 4))
                nc.scalar.activation(
                    ab_sb[:, (tcb % 4) * D:(tcb % 4 + 1) * D],
                    banks[(tcb % 4) * 2][:], AF.Copy).then_inc(s_act, 1)


        @block.vector
        def _(vector):
            # magnitudes + incremental top-8 (interleaved, max8 lags 1 kc)
            def mag_kc(kc):
                vector.wait_ge(s_act, 4 * kc + 2)
                dsl = slice(kc * D, (kc + 1) * D)
                nc.vector.tensor_tensor(mag[:, dsl], r2[:, dsl], r2[:, dsl],
                                        ALU.mult)
                nc.vector.tensor_tensor(sqt[:], i2[:, dsl], i2[:, dsl],
                                        ALU.mult)
                nc.vector.tensor_tensor(mag[:, dsl], mag[:, dsl], sqt[:],
                                        ALU.add).then_inc(s_dve, 1)

            def max8_kc(kc):
                vector.wait_ge(s_pe, _TP(kc))
                b = banks[(kc % 4) * 2]
                for dc in range(NDC):
                    mx = nc.vector.max(
                        out=m8i[:, dc * 64 + kc * 8:dc * 64 + (kc + 1) * 8],
                        in_=b[:, dc * 128:(dc + 1) * 128])
                    if dc == NDC - 1:
                        mx.then_inc(s_dve, 1)

            mag_kc(0)
            for kc in range(1, NKC):
                mag_kc(kc)
                max8_kc(kc - 1)
            max8_kc(NKC - 1)
            for dc in range(NDC):
                mx = nc.vector.max(out=m8f[:, dc * 8:(dc + 1) * 8],
                                   in_=m8i[:, dc * 64:(dc + 1) * 64])
                if dc == NDC - 1:
                    mx.then_inc(s_dve, 1)
            # mask + apply to the pre-cast bf16 coefficients (in place)
            vector.wait_ge(s_act, 31)   # thb
            for kc in range(NKC):
                if kc == NKC - 1:
                    vector.wait_ge(s_act, 33)   # k7 casts
                dsl = slice(kc * D, (kc + 1) * D)
                nc.vector.tensor_tensor(msk[:], mag[:, dsl], thb[:], ALU.is_ge)
                nc.vector.tensor_tensor(r2h[:, dsl], r2h[:, dsl], msk[:],
                                        ALU.mult)
                nc.vector.tensor_tensor(i2h[:, dsl], i2h[:, dsl], msk[:],
                                        ALU.mult).then_inc(s_dve, 1)
            # inverse combines: lo = A+B, hi = A-B (A from sbuf, B from psum)
            for tcb in range(8):
                vector.wait_ge(s_act, 34 + tcb)   # A evicted (implies B done)
                if tcb >= 4:
                    vector.wait_ge(s_out[tcb % 4], 32 * (tcb // 4))
                bB = banks[(tcb % 4) * 2 + 1]
                asl = ab_sb[:, (tcb % 4) * D:(tcb % 4 + 1) * D]
                osl = slice((tcb % 4) * D, (tcb % 4 + 1) * D)
                nc.vector.tensor_tensor(ot_lo[:, osl], asl, bB[:],
                                        ALU.add).then_inc(s_dve, 1)
                nc.vector.tensor_tensor(ot_hi[:, osl], asl, bB[:],
                                        ALU.subtract).then_inc(s_dve, 1)
                if tcb == 0:
                    # out[1024] into ot_hi slot-0 row 0 (from pmrow psum)
                    vector.wait_ge(s_pe, 30)
                    nc.vector.tensor_copy(
                        ot_hi[0:1, 0:D], banks[0][0:1, :]).then_inc(s_dve, 1)


# ---------------- host side ----------------

_BF = ml_dtypes.bfloat16


def _split_hilo(a32):
    hi = a32.astype(_BF)
    lo = (a32 - hi.astype(np.float32)).astype(_BF)
    return hi, lo


def _make_constants():
    t = np.arange(T, dtype=np.float64)[:, None]
    k = np.arange(1, KF + 1, dtype=np.float64)[None, :]
    ang = 2.0 * np.pi * t * k / T
    C = np.cos(ang)
    S = -np.sin(ang)
    C[:, KF - 1] = 0.0
    S[:, KF - 1] = 0.0

    # folded forward halves
    Chalf = np.zeros((NCA * 128, KF))
    Chalf[:TH] = C[:TH]
    Chalf[TH] = np.cos(np.pi * k[0])
    Chalf[TH, KF - 1] = 0.0
    Shalf = np.zeros((NCB * 128, KF))
    Shalf[:] = S[:TH]

    def stripes(m64, ncc):
        hi, lo = _split_hilo(m64.astype(np.float32))
        # [a*128+p, kc*128+u] -> [kc, p, a, {hi|lo}, u]
        def tile(m):
            b = np.asarray(m, dtype=np.float32).reshape(ncc, 128, NKC, 128)
            return b.transpose(2, 1, 0, 3)             # (kc, p, a, u)
        st = np.stack([tile(hi), tile(lo)], axis=3)    # (kc, p, a, 2, u)
        st = st.reshape(NKC, 128, ncc * 256)
        if ncc < NCA:
            pad = np.zeros((NKC, 128, (NCA - ncc) * 256), np.float32)
            st = np.concatenate([st, pad], axis=2)
        return st

    cre = stripes(Chalf, NCA)
    cim = stripes(Shalf, NCB)
    cfc = np.empty((NCF, 128, WRE), np.float32)
    cfc[0::2] = cre
    cfc[1::2] = cim
    cfc = cfc.astype(_BF)

    # inverse blocks (single bf16), t = 0..1023 (row 1024 done on host)
    Ci = C[:TH].astype(np.float32)
    Si = S[:TH].astype(np.float32)

    def blocks(m32):
        M = np.ascontiguousarray(m32.T)                  # (KF, 1024)
        blk = M.reshape(NKC, 128, NIV, 128)              # (kc, p, tc, u)
        blk = np.ascontiguousarray(blk.transpose(2, 1, 0, 3))
        return blk.reshape(NIV, 128, KF)

    ivc = np.ascontiguousarray(
        np.concatenate([blocks(Ci), blocks(Si)], axis=2)).astype(_BF)
    pmc = ((-1.0) ** (np.arange(128) + 1)).astype(_BF)[:, None]
    return dict(cf=np.ascontiguousarray(cfc), iv=ivc, pm=pmc)


_CONSTS = None
LAST_EXEC_NS = None
LAST_RES = None
TRACE = False


def kernel(input_tensor: np.ndarray) -> np.ndarray:
    from concourse.bass_utils import run_bass_kernel_spmd

    global _CONSTS
    if _CONSTS is None:
        _CONSTS = _make_constants()

    x = np.asarray(input_tensor, dtype=np.float32)
    B = x.shape[0]
    assert x.shape == (B, T, D)

    nc = bass.Bass("TRN2", target_bir_lowering=False)
    build_kernel(nc)

    in_maps = []
    for b in range(B):
        xb = x[b].astype(np.float64)
        u = np.zeros((NCA * 128, D))
        v = np.zeros((NCB * 128, D))
        u[0] = xb[0]
        u[1:TH] = xb[1:TH] + xb[T - 1:TH:-1]
        u[TH] = xb[TH]
        v[1:TH] = xb[1:TH] - xb[T - 1:TH:-1]

        def pre(m, ncc):   # [a*128+p, d] -> [p, a*D+d] (contiguous DMA)
            return np.ascontiguousarray(
                m.reshape(ncc, 128, D).transpose(1, 0, 2).reshape(128, ncc * D))

        uh_np, ul_np = _split_hilo(u.astype(np.float32))
        vh_np, vl_np = _split_hilo(v.astype(np.float32))
        in_maps.append({"uh": pre(uh_np, NCA), "ul": pre(ul_np, NCA),
                        "vh": pre(vh_np, NCB), "vl": pre(vl_np, NCB),
                        **_CONSTS})

    global LAST_EXEC_NS, LAST_RES
    res = run_bass_kernel_spmd(nc, in_maps, core_ids=list(range(B)), trace=TRACE)
    LAST_EXEC_NS = res.exec_time_ns
    LAST_RES = res
    outs = []
    for b in range(B):
        y = res.results[b]["out"].astype(np.float32)
        y[TH + 1:] = y[TH + 1:][::-1]   # unreverse the reflected half
        outs.append(y)
    return np.stack(outs, axis=0)


if __name__ == "__main__":
    rng = np.random.default_rng(0)
    x = rng.standard_normal((8, T, D), dtype=np.float32)
    y = kernel(input_tensor=x)
    print("out", y.shape, y.dtype)
